# revision 1
# baseline (speedup 1.0000x reference)
"""Multi-head causal attention (B=4, S=2048, H=1024, NH=16) on 8 trn2 cores.

Head-sharded tensor parallelism: core i computes heads {2i, 2i+1}.  Each core
runs projections for its 2 heads (fp32r matmuls), causal flash-style attention
in a transposed orientation (scores S^T[k,q] so the P@V contraction needs no
transpose of P), and a partial output projection over its 128 channels.  The
8 partial outputs are summed on the host (the tensor-parallel all-reduce),
plus the output bias.
"""
import numpy as np

import concourse.bacc as bacc
import concourse.tile as tile
from concourse import mybir
from concourse.bass_utils import run_bass_kernel_spmd

F32 = mybir.dt.float32
F32R = mybir.dt.float32r
AF = mybir.ActivationFunctionType

B, S, H, NH = 4, 2048, 1024, 16
HD = H // NH            # 64
NCORES = 8
HPC = NH // NCORES      # 2 heads per core
C = HPC * HD            # 128 channels per core
SCALE = 1.0 / np.sqrt(HD)

QT_W = 256              # q-tile width (columns of S^T tiles)
KC = 128                # k-chunk (contraction tile for P@V)
N_QT = S // QT_W        # 8
N_KC = S // KC          # 16
N_HC = H // 128         # 8 contraction chunks for projections
N_ST = 4                # s-tiles of 512 for projections

_CACHE = {}
PHASES = ("proj", "vtrans", "attn", "oproj")
PROJ_PRIO = 0


def _build_nc():
    nc = bacc.Bacc(name="mha_tp")
    xt_d = nc.dram_tensor("xt", [B, H, S], F32R, kind="ExternalInput")
    wq_d = nc.dram_tensor("wqt", [H, C], F32R, kind="ExternalInput")
    wk_d = nc.dram_tensor("wkt", [H, C], F32R, kind="ExternalInput")
    wv_d = nc.dram_tensor("wvt", [H, C], F32R, kind="ExternalInput")
    wo_d = nc.dram_tensor("wot", [C, H], F32R, kind="ExternalInput")
    bq_d = nc.dram_tensor("bq", [C, 1], F32, kind="ExternalInput")
    bk_d = nc.dram_tensor("bk", [C, 1], F32, kind="ExternalInput")
    bv_d = nc.dram_tensor("bv", [C, 1], F32, kind="ExternalInput")
    mk_d = nc.dram_tensor("maskbuf", [128, 896], F32R, kind="ExternalInput")
    id_d = nc.dram_tensor("ident", [128, 128], F32, kind="ExternalInput")
    on_d = nc.dram_tensor("ones16", [128, N_KC], F32R, kind="ExternalInput")
    out_d = nc.dram_tensor("out", [B, S, H], F32, kind="ExternalOutput")

    with tile.TileContext(nc) as tc:
        with (
            tc.tile_pool(name="const", bufs=1) as cp,
            tc.tile_pool(name="big", bufs=2) as bp,
            tc.tile_pool(name="work", bufs=2) as wp,
            tc.tile_pool(name="xs", bufs=12) as xp,
            tc.tile_pool(name="ps", bufs=1, space="PSUM") as ps,
            tc.tile_pool(name="psmix", bufs=2, space="PSUM") as pm,
        ):
            # ---- constants ----
            wq_s = cp.tile([128, H], F32R)
            wk_s = cp.tile([128, H], F32R)
            wv_s = cp.tile([128, H], F32R)
            wo_s = cp.tile([128, H], F32R)
            mk_s = cp.tile([128, 896], F32R)
            id_s = cp.tile([128, 128], F32)
            on_s = cp.tile([128, N_KC], F32R)
            bq_s = cp.tile([C, 1], F32)
            bk_s = cp.tile([C, 1], F32)
            bv_s = cp.tile([C, 1], F32)
            for w_s, w_d in ((wq_s, wq_d), (wk_s, wk_d), (wv_s, wv_d)):
                nc.scalar.dma_start(
                    w_s.rearrange("p (c d) -> p c d", d=128),
                    w_d.ap().rearrange("(c p) d -> p c d", p=128))
            nc.scalar.dma_start(wo_s[:], wo_d.ap())
            nc.scalar.dma_start(mk_s[:], mk_d.ap())
            nc.scalar.dma_start(id_s[:], id_d.ap())
            nc.scalar.dma_start(on_s[:], on_d.ap())
            nc.scalar.dma_start(bq_s[:], bq_d.ap())
            nc.scalar.dma_start(bk_s[:], bk_d.ap())
            nc.scalar.dma_start(bv_s[:], bv_d.ap())

            tiles = {}

            def emit_proj(b, halves=(0, 1)):
                # ---- projections: QT/KT [128, S] f32r, VT [128, S] f32 ----
                if b not in tiles:
                    qt = bp.tile([128, S], F32R, tag="qt", name=f"qt{b}")
                    kt = bp.tile([128, S], F32R, tag="kt", name=f"kt{b}")
                    vt = bp.tile([128, S], F32, tag="vt", name=f"vt{b}", bufs=1)
                    tiles[b] = {"qt": qt, "kt": kt, "vt": vt}
                qt, kt, vt = tiles[b]["qt"], tiles[b]["kt"], tiles[b]["vt"]
                if True:
                  for half in halves if "proj" in PHASES else []:
                    xts = []
                    for hc in range(N_HC):
                        hsl = slice(hc * 128, (hc + 1) * 128)
                        xt_t = xp.tile([128, 1024], F32R, tag="xt",
                                       name=f"x{b}_{half}_{hc}")
                        nc.sync.dma_start(
                            xt_t[:], xt_d.ap()[b, hsl, half * 1024:(half + 1) * 1024])
                        xts.append(xt_t)
                    for sth in range(2):
                        st = half * 2 + sth
                        ssl = slice(st * 512, (st + 1) * 512)
                        # sequential Q/K/V passes over resident x^T chunks: 2
                        # PSUM slots suffice (pipeline pass i+1 against copy i)
                        for w_s, bias, dst, pnm in ((wq_s, bq_s, qt, "q"),
                                                    (wk_s, bk_s, kt, "k"),
                                                    (wv_s, bv_s, vt, "v")):
                            pp = pm.tile([128, 512], F32, tag="mix",
                                         name=f"pp{pnm}{b}_{st}")
                            for hc in range(N_HC):
                                nc.tensor.matmul(
                                    pp[:], w_s[:, hc * 128:(hc + 1) * 128],
                                    xts[hc][:, sth * 512:(sth + 1) * 512],
                                    start=(hc == 0), stop=(hc == N_HC - 1))
                            nc.vector.tensor_scalar_add(dst[:, ssl], pp[:], bias[:])

            def emit_vtrans(b):
                # ---- V transpose: vn_h [128, 16*65] (ones col at 64 of each 65) ----
                vt = tiles[b]["vt"]
                vna = bp.tile([128, N_KC * (HD + 1)], F32R, tag="vna", name=f"vna{b}")
                vnb = bp.tile([128, N_KC * (HD + 1)], F32R, tag="vnb", name=f"vnb{b}")
                tiles[b]["vna"], tiles[b]["vnb"] = vna, vnb
                for h, vn in ((0, vna), (1, vnb)):
                    vn3 = vn.rearrange("p (c e) -> p c e", e=HD + 1)
                    nc.sync.dma_start(vn3[:, :, HD], on_d.ap())
                for c in range(N_KC) if "vtrans" in PHASES else []:
                    tp = pm.tile([128, 128], F32, tag="mix", name=f"tp{b}_{c}")
                    nc.tensor.transpose(tp[:], vt[:, c * 128:(c + 1) * 128], id_s[:])
                    nc.any.tensor_copy(vna[:, c * (HD + 1): c * (HD + 1) + HD],
                                       tp[:, 0:HD])
                    nc.any.tensor_copy(vnb[:, c * (HD + 1): c * (HD + 1) + HD],
                                       tp[:, HD:2 * HD])

            def emit_attn(b, jlo=0, jhi=N_QT):
                # ---- attention (transposed scores), both heads interleaved ----
                qt, kt = tiles[b]["qt"], tiles[b]["kt"]
                if "ctx" not in tiles[b]:
                    ctx = bp.tile([128, S], F32R, tag="ctx", name=f"ctx{b}")
                    tiles[b]["ctx"] = ctx
                ctx = tiles[b]["ctx"]
                vns = (tiles[b]["vna"], tiles[b]["vnb"])
                for j in range(jlo, jhi) if "attn" in PHASES else []:
                    qsl = slice(j * QT_W, (j + 1) * QT_W)
                    acc = ps.tile([128, 512], F32, tag="acc", name=f"acc{b}_{j}",
                                  bufs=2)
                    nc.vector.memset(acc[:], 0.0)
                    nkc = 2 * (j + 1)              # causal: k-chunks 0..nkc-1
                    n_sc = (nkc + 3) // 4
                    for sc in range(n_sc):
                        cs = [c for c in range(4 * sc, min(4 * sc + 4, nkc))]
                        sts, pts = [], []
                        for h in range(2):
                            st_h = ps.tile([128, 4 * QT_W], F32, tag=f"st{h}",
                                           name=f"st{h}_{b}_{j}_{sc}")
                            pt_h = wp.tile([128, 4 * QT_W], F32R, tag=f"pt{h}",
                                           name=f"pt{h}_{b}_{j}_{sc}", bufs=5)
                            sts.append(st_h)
                            pts.append(pt_h)
                        for c in cs:   # QK: heads adjacent -> row-group concurrency
                            for h in range(2):
                                hsl = slice(h * HD, (h + 1) * HD)
                                nc.tensor.matmul(
                                    sts[h][:, (c - 4 * sc) * QT_W:(c - 4 * sc + 1) * QT_W],
                                    kt[hsl, c * KC:(c + 1) * KC],
                                    qt[hsl, qsl],
                                    start=True, stop=True,
                                )
                        w = len(cs) * QT_W
                        for h in range(2):
                            nc.scalar.activation(pts[h][:, 0:w], sts[h][:, 0:w],
                                                 AF.Exp, scale=float(SCALE))
                        if sc == n_sc - 1:  # diagonal: mask last two k-chunks
                            for h in range(2):
                                for c in (nkc - 2, nkc - 1):
                                    mo = 384 - 128 * (c - 2 * j)  # o = 128*(c-2j)
                                    nc.gpsimd.tensor_mul(
                                        pts[h][:, (c - 4 * sc) * QT_W:(c - 4 * sc + 1) * QT_W],
                                        pts[h][:, (c - 4 * sc) * QT_W:(c - 4 * sc + 1) * QT_W],
                                        mk_s[:, mo:mo + QT_W],
                                    )
                        for c in cs:   # P@V (+ones rowsum row)
                            for h in range(2):
                                nc.tensor.matmul(
                                    acc[0:HD + 1, h * QT_W:(h + 1) * QT_W],
                                    vns[h][:, c * (HD + 1):(c + 1) * (HD + 1)],
                                    pts[h][:, (c - 4 * sc) * QT_W:(c - 4 * sc + 1) * QT_W],
                                    start=False, stop=(c == nkc - 1),
                                    skip_group_check=True,
                                )
                    # normalize: one recip over both heads' rowsum halves,
                    # partition-broadcast on the (idle) gpsimd, one fused mul
                    recip = wp.tile([1, 2 * QT_W], F32, tag="recip",
                                    name=f"rc{b}_{j}")
                    nc.vector.reciprocal(recip[:], acc[HD:HD + 1, :])
                    for h in range(2):
                        asl = slice(h * QT_W, (h + 1) * QT_W)
                        bc_sb = wp.tile([HD, QT_W], F32, tag="bcs",
                                        name=f"bcs{b}_{j}_{h}", bufs=4)
                        nc.gpsimd.partition_broadcast(bc_sb[:], recip[0:1, asl])
                        nc.any.tensor_mul(ctx[h * HD:(h + 1) * HD, qsl],
                                          acc[0:HD, asl], bc_sb[:])

            def emit_oproj(b):
                ctx = tiles[b]["ctx"]
                for qp in range(S // 256) if "oproj" in PHASES else []:
                    osb = wp.tile([128, 2048], F32, tag="osb", name=f"ob{b}_{qp}")
                    for sub in range(2):
                        qc = 2 * qp + sub
                        for half in range(2):
                            osl = slice(half * 512, (half + 1) * 512)
                            op = pm.tile([128, 512], F32, tag="mix",
                                         name=f"op{b}_{qc}_{half}")
                            nc.tensor.matmul(op[:], ctx[:, qc * 128:(qc + 1) * 128],
                                             wo_s[:, osl], start=True, stop=True)
                            nc.vector.tensor_copy(
                                osb[:, sub * 1024 + half * 512:
                                    sub * 1024 + (half + 1) * 512], op[:])
                    nc.sync.dma_start(
                        out_d.ap()[b, qp * 256:(qp + 1) * 256, :]
                        .rearrange("(g q) o -> q g o", g=2),
                        osb.rearrange("p (g o) -> p g o", g=2))

            # software-pipelined emission: batch b+1's projection halves are
            # interleaved into batch b's (ACT-gated) attention j-loop so PE
            # always has prioritized fill work; the heavier fill (half 1 +
            # V-transpose) lands before the large causal j-tiles
            emit_proj(0)
            emit_vtrans(0)
            for b in range(B):
                if b + 1 < B:
                    emit_proj(b + 1, halves=(0,))
                emit_attn(b, 0, 4)
                if b + 1 < B:
                    emit_proj(b + 1, halves=(1,))
                    emit_vtrans(b + 1)
                emit_attn(b, 4, N_QT)
                emit_oproj(b)

                # ---- output projection (partial over this core's channels) ----

    nc.compile()
    return nc


def _get_nc():
    if "nc" not in _CACHE:
        _CACHE["nc"] = _build_nc()
    return _CACHE["nc"]


def make_in_maps(x, Wq, bq, Wk, bk, Wv, bv, Wo):
    """Host-side sharding: returns per-core input dicts."""
    xt = np.ascontiguousarray(np.transpose(np.asarray(x, np.float32), (0, 2, 1)))
    mask = (np.arange(896, dtype=np.int64)[None, :]
            >= (np.arange(128, dtype=np.int64)[:, None] + 384)).astype(np.float32)
    ident = np.eye(128, dtype=np.float32)
    ones16 = np.ones((128, N_KC), dtype=np.float32)
    in_maps = []
    for i in range(NCORES):
        r = slice(i * C, (i + 1) * C)
        in_maps.append({
            "xt": xt,
            "wqt": np.ascontiguousarray(np.asarray(Wq, np.float32)[r, :].T),
            "wkt": np.ascontiguousarray(np.asarray(Wk, np.float32)[r, :].T),
            "wvt": np.ascontiguousarray(np.asarray(Wv, np.float32)[r, :].T),
            "wot": np.ascontiguousarray(np.asarray(Wo, np.float32)[:, r].T),
            "bq": np.asarray(bq, np.float32)[r].reshape(C, 1),
            "bk": np.asarray(bk, np.float32)[r].reshape(C, 1),
            "bv": np.asarray(bv, np.float32)[r].reshape(C, 1),
            "maskbuf": mask,
            "ident": ident,
            "ones16": ones16,
        })
    return in_maps


def run_cores(in_maps):
    nc = _get_nc()
    res = run_bass_kernel_spmd(nc, in_maps, core_ids=list(range(NCORES)))
    return [r["out"] for r in res.results]


def kernel(x, mask, Wq, bq, Wk, bk, Wv, bv, Wo, bo):
    in_maps = make_in_maps(x, Wq, bq, Wk, bk, Wv, bv, Wo)
    partials = run_cores(in_maps)
    out = partials[0]
    for p in partials[1:]:
        out = out + p
    return (out + np.asarray(bo, np.float32)[None, None, :]).astype(np.float32)



# revision 36
# speedup vs baseline: 1.4827x; 1.4827x over previous
"""Multi-head causal attention (B=4, S=2048, H=1024, NH=16) on 8 trn2 cores.

Sharding: core = (batch b, head-group g) with 4 batches x 2 groups; each core
computes 8 heads of one batch.  Host sums the 2 group partials per batch and
adds the output bias.

Per-core pipeline (dtype plan driven by fp8-e4m3 error measurements):
 - Q/K projections in fp8 DoubleRow (x split hi+lo planes on host for error
   compensation; weights single-quantized, duplicated across planes).
 - V^T computed directly via orientation swap (x^T chunks stationary, Wv
   moving) in bf16 -- no on-device V transpose.
 - Scores S^T[k,q] per head via fp8 DoubleRow with K compensated hi/lo planes
   and Q broadcast across planes.
 - exp on ACT -> P bf16 tiles; causal mask multiplied on the diagonal chunks.
 - P@V with P as the *stationary* operand [k,q] and V^T [k,64+ones] moving:
   full PE rate in bf16, rowsum for free; per-partition normalize.
 - ctx^T -> ctx via PE transpose; output projection bf16; bf16 partial out.
"""
import numpy as np
import ml_dtypes

import concourse.bacc as bacc
import concourse.tile as tile
from concourse import mybir
from concourse.bass_utils import run_bass_kernel_spmd

F32 = mybir.dt.float32
BF16 = mybir.dt.bfloat16
FP8 = mybir.dt.float8e4
AF = mybir.ActivationFunctionType
DR = mybir.MatmulPerfMode.DoubleRow
MUL = mybir.AluOpType.mult
ADD = mybir.AluOpType.add
SUB = mybir.AluOpType.subtract

B, S, H, NH = 4, 2048, 1024, 16
HD = H // NH            # 64
NCORES = 8
HPC = 8                 # heads per core
C = HPC * HD            # 512 channels per core
SCALE = 1.0 / np.sqrt(HD)
N_QC = S // 128         # 16 q-chunks
N_KT = S // 128         # 16 k-tiles
N_HC = H // 128         # 8 hidden chunks
N_CB = C // 128         # 4 channel blocks
N_ST = 4                # seq tiles of 512 for Q/K proj

_CACHE = {}


def _build_nc():
    nc = bacc.Bacc(name="mha_tp2")
    x8_d = nc.dram_tensor("x8", [128, N_HC, 2, S], FP8, kind="ExternalInput")
    x16_d = nc.dram_tensor("x16", [128, N_HC, S], BF16, kind="ExternalInput")
    wq8_d = nc.dram_tensor("wq8", [128, N_HC, C], FP8, kind="ExternalInput")
    wk8_d = nc.dram_tensor("wk8", [128, N_HC, C], FP8, kind="ExternalInput")
    wvt_d = nc.dram_tensor("wvt", [128, N_HC, C], BF16, kind="ExternalInput")
    wo_d = nc.dram_tensor("wo", [128, N_CB, H], BF16, kind="ExternalInput")
    bq_d = nc.dram_tensor("bq", [128, N_CB], F32, kind="ExternalInput")
    bk_d = nc.dram_tensor("bk", [128, N_CB], F32, kind="ExternalInput")
    vb_d = nc.dram_tensor("vb", [128, HPC, HD], BF16, kind="ExternalInput")
    ut_d = nc.dram_tensor("ut", [128, 128], BF16, kind="ExternalInput")
    id_d = nc.dram_tensor("idb", [128, 128], BF16, kind="ExternalInput")
    out_d = nc.dram_tensor("out", [S, H], BF16, kind="ExternalOutput")

    with tile.TileContext(nc) as tc:
        with (
            tc.tile_pool(name="const", bufs=1) as cp,
            tc.tile_pool(name="work", bufs=2) as wp,
            tc.tile_pool(name="psA", bufs=2, space="PSUM") as psA,
            tc.tile_pool(name="psB", bufs=2, space="PSUM") as psB,
        ):
            # ---- persistent SBUF ----
            x8_s = cp.tile([128, N_HC, 2, S], FP8)
            x16_s = cp.tile([128, N_HC, S], BF16)
            wq8_s = cp.tile([128, N_HC, C], FP8)
            wk8_s = cp.tile([128, N_HC, C], FP8)
            wvt_s = cp.tile([128, N_HC, C], BF16)
            wo_s = cp.tile([128, N_CB, H], BF16)
            bq_s = cp.tile([128, N_CB], F32)
            bk_s = cp.tile([128, N_CB], F32)
            vb_s = cp.tile([128, HPC, HD], BF16)
            ut_s = cp.tile([128, 128], BF16)
            id_s = cp.tile([128, 128], BF16)
            z1_s = cp.tile([1, 128], BF16)
            z2_s = cp.tile([1, 512], BF16)
            q8_s = cp.tile([128, N_CB, S], FP8)
            k8_s = cp.tile([128, N_CB, 2, S], FP8)
            q8o_s = cp.tile([64, N_CB, S], FP8)
            k8o_s = cp.tile([64, N_CB, 2, S], FP8)
            vt_s = cp.tile([128, N_KT, HPC, HD + 1], BF16)
            ctx_s = cp.tile([128, N_CB, S], BF16)

            # DMA order tuned for ramp: small consts, QK weights, first x8
            # chunk (so projections start ~6us in), then the rest.
            for w_s, w_d in ((bq_s, bq_d), (bk_s, bk_d), (vb_s, vb_d),
                             (ut_s, ut_d), (id_s, id_d),
                             (wk8_s, wk8_d)):
                nc.sync.dma_start(w_s[:], w_d.ap())
            nc.vector.memset(vt_s[:, :, :, HD:HD + 1], 1.0)
            nc.vector.memset(z1_s[:], 0.0)
            nc.vector.memset(z2_s[:], 0.0)

            def dma_x8(lo, hi):
                ssl = slice(lo * 128, hi * 128)
                nc.sync.dma_start(x8_s[:, :, :, ssl], x8_d.ap()[:, :, :, ssl])

            def dma_x16(lo, hi):
                ssl = slice(lo * 128, hi * 128)
                nc.sync.dma_start(x16_s[:, :, ssl], x16_d.ap()[:, :, ssl])

            dma_x8(0, 4)
            nc.sync.dma_start(wq8_s[:], wq8_d.ap())
            dma_x16(0, 4)
            nc.sync.dma_start(wvt_s[:], wvt_d.ap())
            dma_x8(4, 8)
            nc.sync.dma_start(wo_s[:], wo_d.ap())
            dma_x16(4, 8)
            dma_x8(8, 12)
            dma_x16(8, 12)
            dma_x8(12, 16)
            dma_x16(12, 16)

            def emit_projqk_unit(st, which, cb):
                # one (512-seq-tile, weight, chan-block) projection unit
                ssl = slice(st * 512, (st + 1) * 512)
                w_s, dst, b_s, comp = (
                    (wk8_s, k8_s, bk_s, True) if which == "k"
                    else (wq8_s, q8_s, bq_s, False))
                pp = psB.tile([128, 512], F32, tag="op", bufs=2,
                              name=f"pp{which}{st}_{cb}")
                for hc in range(N_HC):
                    nc.tensor.matmul(
                        pp[:],
                        w_s[:, hc, cb * 128:(cb + 1) * 128].unsqueeze(1)
                            .broadcast_to([128, 2, 128]),
                        x8_s[:, hc, :, ssl],
                        start=(hc == 0), stop=(hc == N_HC - 1),
                        perf_mode=DR)
                if comp:
                    nc.vector.tensor_scalar_add(
                        dst[:, cb, 0, ssl], pp[:], b_s[:, cb:cb + 1])
                    nc.vector.scalar_tensor_tensor(
                        dst[:, cb, 1, ssl], pp[:], b_s[:, cb:cb + 1],
                        dst[:, cb, 0, ssl], ADD, SUB)
                    nc.sync.dma_start(k8o_s[:, cb, :, ssl],
                                      dst[64:128, cb, :, ssl])
                else:
                    nc.vector.tensor_scalar_add(
                        dst[:, cb, ssl], pp[:], b_s[:, cb:cb + 1])
                    nc.sync.dma_start(q8o_s[:, cb, ssl],
                                      dst[64:128, cb, ssl])

            def emit_projv(kt):
                # V^T tile for k positions [kt*128, (kt+1)*128)
                pp = psA.tile([128, 512], F32, tag="sc", name=f"pv{kt}")
                for hc in range(N_HC):
                    nc.tensor.matmul(
                        pp[:], x16_s[:, hc, kt * 128:(kt + 1) * 128],
                        wvt_s[:, hc, :],
                        start=(hc == 0), stop=(hc == N_HC - 1))
                nc.vector.tensor_tensor(
                    vt_s[:, kt, :, 0:HD],
                    pp[:].rearrange("p (h d) -> p h d", h=HPC), vb_s[:], ADD)

            # ---- attention ----
            # per (qc, kc): QK for 8 heads -> one exp -> 8 PV matmuls.
            # software pipelined one block deep: PV(block i) is emitted after
            # QK(block i+1) so exp(i) overlaps PE work.  PE idle inside the
            # ACT-bound attention loop is filled from a queue of projection /
            # output-projection units.
            pend_q = []   # [(p_tile, kc, qc, accs)] pipeline, depth 2
            acc_of = {}
            fillers = []  # (needed_by_row, emit_fn)

            def emit_norm(qc):
                accs = acc_of.pop(qc)
                ctxT = wp.tile([128, HPC, HD], BF16, tag="ctxT", name=f"cT{qc}")
                for i, a in enumerate(accs):
                    av = a[:, 0:4 * 65].rearrange("p (h e) -> p h e", e=65)
                    denr = wp.tile([128, 4], F32, tag="denr", bufs=4,
                                   name=f"dn{qc}_{i}")
                    nc.vector.reciprocal(denr[:], av[:, :, HD])
                    nc.vector.tensor_tensor(
                        ctxT[:, 4 * i:4 * i + 4, :], av[:, :, 0:HD],
                        denr[:].unsqueeze(2).broadcast_to([128, 4, HD]), MUL)
                # transpose [q, c] -> [c, q] on the DMA crossbar (off PE)
                nc.sync.dma_start_transpose(
                    ctx_s[:, :, qc * 128:(qc + 1) * 128], ctxT[:])
                for oh in range(2):
                    fillers.append((qc + 4, 900.0,
                                    lambda qc=qc, oh=oh: emit_oproj(qc, oh)))

            def flush_pv():
                if not pend_q:
                    return
                p_t, kc, qc, accs = pend_q.pop(0)
                for h in range(HPC):
                    a = accs[h // 4]
                    nc.tensor.matmul(
                        a[:, 65 * (h % 4):65 * (h % 4) + 65],
                        p_t[:, h, :], vt_s[:, kc, h, :],
                        start=False, stop=(kc == qc),
                        skip_group_check=True)
                if kc == qc:
                    emit_norm(qc)

            def emit_qk(qc, kc):
                if kc == 0:
                    accA = psB.tile([128, 512], F32, tag="accA", bufs=1,
                                    name=f"aA{qc}")
                    accB = psB.tile([128, 512], F32, tag="accB", bufs=1,
                                    name=f"aB{qc}")
                    for a in (accA, accB):
                        nc.tensor.matmul(a[:], z1_s[:], z2_s[:],
                                         start=True, stop=True)
                    acc_of[qc] = (accA, accB)
                accs = acc_of[qc]
                sc_t = psA.tile([128, HPC, 128], F32, tag="sc",
                                name=f"s{qc}_{kc}")
                qsl = slice(qc * 128, (qc + 1) * 128)
                ksl = slice(kc * 128, (kc + 1) * 128)
                for h in range(HPC):
                    cb = h // 2
                    if h % 2 == 0:
                        k_ap = k8_s[0:64, cb, :, ksl]
                        q_ap = q8_s[0:64, cb, qsl]
                    else:
                        k_ap = k8o_s[:, cb, :, ksl]
                        q_ap = q8o_s[:, cb, qsl]
                    nc.tensor.matmul(
                        sc_t[:, h, :], k_ap,
                        q_ap.unsqueeze(1).broadcast_to([64, 2, 128]),
                        start=True, stop=True, perf_mode=DR)
                p_t = wp.tile([128, HPC, 128], BF16, tag="p", bufs=4,
                              name=f"p{qc}_{kc}")
                nc.scalar.activation(p_t[:], sc_t[:], AF.Exp, scale=float(SCALE))
                if kc == qc:
                    nc.vector.tensor_tensor(
                        p_t[:], p_t[:],
                        ut_s[:].unsqueeze(1).broadcast_to([128, HPC, 128]), MUL)
                pend_q.append((p_t, kc, qc, accs))

            ob_of = {}

            def emit_oproj(sc, oh):
                ssl = slice(sc * 128, (sc + 1) * 128)
                if sc not in ob_of:
                    ob_of[sc] = wp.tile([128, H], BF16, tag="ob",
                                        name=f"ob{sc}")
                ob = ob_of[sc]
                osl = slice(oh * 512, (oh + 1) * 512)
                op = psB.tile([128, 512], F32, tag="op", name=f"o{sc}_{oh}")
                for cb in range(N_CB):
                    nc.tensor.matmul(op[:], ctx_s[:, cb, ssl],
                                     wo_s[:, cb, osl],
                                     start=(cb == 0), stop=(cb == N_CB - 1))
                nc.vector.tensor_copy(ob[:, osl], op[:])
                if oh == 1:
                    nc.sync.dma_start(out_d.ap()[ssl, :], ob[:])
                    del ob_of[sc]

            fill_ns = [0.0]     # filler PE-time emitted so far
            blocks = [0]        # attention blocks emitted so far
            SLACK_NS = 700.0    # PE slack per ACT-bound block

            def pop_fillers(row, opportunistic):
                # emit everything required by this row; then, if opportunistic,
                # fillers up to the cumulative PE-slack budget
                i = 0
                while i < len(fillers):
                    nb, cost, fn = fillers[i]
                    if nb <= row:
                        fillers.pop(i)
                        fill_ns[0] += cost
                        fn()
                    else:
                        i += 1
                while (opportunistic and fillers
                       and fill_ns[0] < blocks[0] * SLACK_NS):
                    nb, cost, fn = fillers.pop(0)
                    fill_ns[0] += cost
                    fn()

            # ---- emission schedule ----
            for st in range(N_ST):
                for cb in range(N_CB):
                    fillers.append((4 * st, 900.0, lambda st=st, cb=cb:
                                    emit_projqk_unit(st, "k", cb)))
                    fillers.append((4 * st, 900.0, lambda st=st, cb=cb:
                                    emit_projqk_unit(st, "q", cb)))
            for kt in range(N_KT):
                fillers.append((kt, 1750.0, lambda kt=kt: emit_projv(kt)))
            fillers.sort(key=lambda f: f[0])

            for qc in range(N_QC):
                pop_fillers(qc, False)   # force units this row needs
                for kc in range(qc + 1):
                    emit_qk(qc, kc)
                    blocks[0] += 1
                    if len(pend_q) > 2:
                        flush_pv()
                    pop_fillers(qc, True)
            while pend_q:
                flush_pv()
            pop_fillers(999, False)
            while fillers:
                fillers.pop(0)[2]()

    nc.compile()
    return nc


def _get_nc():
    if "nc" not in _CACHE:
        _CACHE["nc"] = _build_nc()
    return _CACHE["nc"]


def _e4(a):
    return a.astype(ml_dtypes.float8_e4m3)


def make_in_maps(x, Wq, bq, Wk, bk, Wv, bv, Wo):
    x = np.asarray(x, np.float32)
    in_maps = []
    ut = np.triu(np.ones((128, 128), np.float32)).astype(ml_dtypes.bfloat16)
    idb = np.eye(128, dtype=ml_dtypes.bfloat16)
    for core in range(NCORES):
        b, g = core // 2, core % 2
        csl = slice(g * C, (g + 1) * C)
        xT = np.ascontiguousarray(x[b].T)                       # [H, S]
        x_hi = _e4(xT)
        x_lo = _e4(xT - x_hi.astype(np.float32))
        x8 = np.stack([x_hi.reshape(N_HC, 128, S),
                       x_lo.reshape(N_HC, 128, S)], axis=2)     # [hc,128,2,S]
        x8 = np.ascontiguousarray(x8.transpose(1, 0, 2, 3))     # [128,hc,2,S]
        x16 = np.ascontiguousarray(
            xT.astype(ml_dtypes.bfloat16).reshape(N_HC, 128, S)
            .transpose(1, 0, 2))
        def wlayout(W):
            W8 = _e4(np.asarray(W, np.float32)[csl, :].T)       # [H, C]
            return np.ascontiguousarray(
                W8.reshape(N_HC, 128, C).transpose(1, 0, 2))
        wvt = (np.asarray(Wv, np.float32)[csl, :].T
               .astype(ml_dtypes.bfloat16).reshape(N_HC, 128, C)
               .transpose(1, 0, 2))
        wo = (np.asarray(Wo, np.float32)[:, csl].T
              .astype(ml_dtypes.bfloat16).reshape(N_CB, 128, H)
              .transpose(1, 0, 2))
        vb = np.broadcast_to(
            np.asarray(bv, np.float32)[csl].reshape(HPC, HD), (128, HPC, HD))
        in_maps.append({
            "x8": x8,
            "x16": np.ascontiguousarray(x16),
            "wq8": wlayout(Wq),
            "wk8": wlayout(Wk),
            "wvt": np.ascontiguousarray(wvt),
            "wo": np.ascontiguousarray(wo),
            "bq": np.ascontiguousarray(
                np.asarray(bq, np.float32)[csl].reshape(N_CB, 128).T),
            "bk": np.ascontiguousarray(
                np.asarray(bk, np.float32)[csl].reshape(N_CB, 128).T),
            "vb": np.ascontiguousarray(vb.astype(ml_dtypes.bfloat16)),
            "ut": ut,
            "idb": idb,
        })
    return in_maps


def unshard(partials, bo):
    """partials: list of 8 [S, H] bf16 arrays -> full [B, S, H] f32."""
    out = np.zeros((B, S, H), np.float32)
    for core, p in enumerate(partials):
        out[core // 2] += np.asarray(p, dtype=np.float32)
    return out + np.asarray(bo, np.float32)[None, None, :]


def kernel(x, mask, Wq, bq, Wk, bk, Wv, bv, Wo, bo):
    nc = _get_nc()
    in_maps = make_in_maps(x, Wq, bq, Wk, bk, Wv, bv, Wo)
    res = run_bass_kernel_spmd(nc, in_maps, core_ids=list(range(NCORES)))
    return unshard([r["out"] for r in res.results], bo).astype(np.float32)


if __name__ == "__main__":
    nc = _get_nc()
    from concourse.timeline_sim import TimelineSim
    print("sim ns:", TimelineSim(nc, trace=False).simulate())


# revision 60
# speedup vs baseline: 1.5631x; 1.0542x over previous
"""Multi-head causal attention (B=4, S=2048, H=1024, NH=16) on 8 trn2 cores.

Sharding: core = (batch b, head-group g) with 4 batches x 2 groups; each core
computes 8 heads of one batch.  Host sums the 2 group partials per batch and
adds the output bias.

Per-core pipeline (dtype plan driven by fp8-e4m3 error measurements):
 - Q/K projections in fp8 DoubleRow (x split hi+lo planes on host for error
   compensation; weights single-quantized, duplicated across planes).
 - V^T computed directly via orientation swap (x^T chunks stationary, Wv
   moving) in bf16 -- no on-device V transpose.
 - Scores S^T[k,q] per head via fp8 DoubleRow with K compensated hi/lo planes
   and Q broadcast across planes.
 - exp on ACT -> P bf16 tiles; causal mask multiplied on the diagonal chunks.
 - P@V with P as the *stationary* operand [k,q] and V^T [k,64+ones] moving:
   full PE rate in bf16, rowsum for free; per-partition normalize.
 - ctx^T -> ctx via PE transpose; output projection bf16; bf16 partial out.
"""
import numpy as np
import ml_dtypes

import concourse.bacc as bacc
import concourse.tile as tile
from concourse import mybir
from concourse.bass_utils import run_bass_kernel_spmd

F32 = mybir.dt.float32
BF16 = mybir.dt.bfloat16
FP8 = mybir.dt.float8e4
AF = mybir.ActivationFunctionType
DR = mybir.MatmulPerfMode.DoubleRow
MUL = mybir.AluOpType.mult
ADD = mybir.AluOpType.add
SUB = mybir.AluOpType.subtract

B, S, H, NH = 4, 2048, 1024, 16
HD = H // NH            # 64
NCORES = 8
HPC = 8                 # heads per core
C = HPC * HD            # 512 channels per core
SCALE = 1.0 / np.sqrt(HD)
N_QC = S // 128         # 16 q-chunks
N_KT = S // 128         # 16 k-tiles
N_HC = H // 128         # 8 hidden chunks
N_CB = C // 128         # 4 channel blocks
N_ST = 4                # seq tiles of 512 for Q/K proj

_CACHE = {}


def _build_nc():
    nc = bacc.Bacc(name="mha_tp2")
    x8_d = nc.dram_tensor("x8", [128, N_HC, 2, S], FP8, kind="ExternalInput")
    x16_d = nc.dram_tensor("x16", [128, N_HC, S], BF16, kind="ExternalInput")
    wq8_d = nc.dram_tensor("wq8", [128, N_HC, C], FP8, kind="ExternalInput")
    wk8_d = nc.dram_tensor("wk8", [128, N_HC, C], FP8, kind="ExternalInput")
    wvt_d = nc.dram_tensor("wvt", [128, N_HC, C], BF16, kind="ExternalInput")
    wo_d = nc.dram_tensor("wo", [128, N_CB, H], BF16, kind="ExternalInput")
    bq_d = nc.dram_tensor("bq", [128, N_CB], F32, kind="ExternalInput")
    bk_d = nc.dram_tensor("bk", [128, N_CB], F32, kind="ExternalInput")
    vb_d = nc.dram_tensor("vb", [128, HPC, HD], BF16, kind="ExternalInput")
    ut_d = nc.dram_tensor("ut", [128, 128], BF16, kind="ExternalInput")
    id_d = nc.dram_tensor("idb", [128, 128], BF16, kind="ExternalInput")
    out_d = nc.dram_tensor("out", [S, H], BF16, kind="ExternalOutput")

    with tile.TileContext(nc) as tc:
        with (
            tc.tile_pool(name="const", bufs=1) as cp,
            tc.tile_pool(name="work", bufs=2) as wp,
            tc.tile_pool(name="psA", bufs=2, space="PSUM") as psA,
            tc.tile_pool(name="psB", bufs=2, space="PSUM") as psB,
        ):
            # ---- persistent SBUF ----
            x8_s = cp.tile([128, N_HC, 2, S], FP8)
            x16_s = cp.tile([128, N_HC, S], BF16)
            wq8_s = cp.tile([128, N_HC, C], FP8)
            wk8_s = cp.tile([128, N_HC, C], FP8)
            wvt_s = cp.tile([128, N_HC, C], BF16)
            wo_s = cp.tile([128, N_CB, H], BF16)
            bq_s = cp.tile([128, N_CB], F32)
            bk_s = cp.tile([128, N_CB], F32)
            vb_s = cp.tile([128, HPC, HD], BF16)
            ut_s = cp.tile([128, 128], BF16)
            id_s = cp.tile([128, 128], BF16)
            z1_s = cp.tile([1, 128], BF16)
            z2_s = cp.tile([1, 512], BF16)
            q8_s = cp.tile([128, N_CB, S], FP8)
            k8_s = cp.tile([128, N_CB, 2, S], FP8)
            q8o_s = cp.tile([64, N_CB, S], FP8)
            k8o_s = cp.tile([64, N_CB, 2, S], FP8)
            vt_s = cp.tile([128, N_KT, HPC, HD + 1], BF16)
            ctx_s = cp.tile([128, N_CB, S], BF16)

            # DMA order tuned for ramp: K weights + first x8 chunk first so
            # the first projection units can start ~5us in.
            nc.sync.dma_start(wk8_s[:], wk8_d.ap())
            nc.vector.memset(vt_s[:, :, :, HD:HD + 1], 1.0)
            nc.vector.memset(z1_s[:], 0.0)
            nc.vector.memset(z2_s[:], 0.0)

            def dma_x8(lo, hi):
                ssl = slice(lo * 128, hi * 128)
                nc.sync.dma_start(x8_s[:, :, :, ssl], x8_d.ap()[:, :, :, ssl])

            def dma_x16(lo, hi):
                ssl = slice(lo * 128, hi * 128)
                nc.sync.dma_start(x16_s[:, :, ssl], x16_d.ap()[:, :, ssl])

            dma_x8(0, 4)
            for w_s, w_d in ((bq_s, bq_d), (bk_s, bk_d), (vb_s, vb_d),
                             (ut_s, ut_d), (id_s, id_d)):
                nc.sync.dma_start(w_s[:], w_d.ap())
            nc.sync.dma_start(wq8_s[:], wq8_d.ap())
            dma_x16(0, 4)
            nc.sync.dma_start(wvt_s[:], wvt_d.ap())
            # the rest of the loads are emitted just-in-time (as fillers on
            # the gpsimd SWDGE queue) so the shared DMA FIFO stays shallow
            # and attention-critical transfers are not stuck behind them.
            late_dmas = [
                (1, lambda: nc.sync.dma_start(wo_s[:], wo_d.ap())),
                (1, lambda: nc.sync.dma_start(
                    x8_s[:, :, :, 512:1024], x8_d.ap()[:, :, :, 512:1024])),
                (1, lambda: nc.sync.dma_start(
                    x16_s[:, :, 512:1024], x16_d.ap()[:, :, 512:1024])),
                (4, lambda: nc.sync.dma_start(
                    x8_s[:, :, :, 1024:1536], x8_d.ap()[:, :, :, 1024:1536])),
                (4, lambda: nc.sync.dma_start(
                    x16_s[:, :, 1024:1536], x16_d.ap()[:, :, 1024:1536])),
                (7, lambda: nc.sync.dma_start(
                    x8_s[:, :, :, 1536:2048], x8_d.ap()[:, :, :, 1536:2048])),
                (7, lambda: nc.sync.dma_start(
                    x16_s[:, :, 1536:2048], x16_d.ap()[:, :, 1536:2048])),
            ]

            def emit_projqk_unit(st, which, cb):
                # one (512-seq-tile, weight, chan-block) projection unit
                ssl = slice(st * 512, (st + 1) * 512)
                w_s, dst, b_s, comp = (
                    (wk8_s, k8_s, bk_s, True) if which == "k"
                    else (wq8_s, q8_s, bq_s, False))
                pp = psB.tile([128, 512], F32, tag="op", bufs=2,
                              name=f"pp{which}{st}_{cb}")
                for hc in range(N_HC):
                    nc.tensor.matmul(
                        pp[:],
                        w_s[:, hc, cb * 128:(cb + 1) * 128].unsqueeze(1)
                            .broadcast_to([128, 2, 128]),
                        x8_s[:, hc, :, ssl],
                        start=(hc == 0), stop=(hc == N_HC - 1),
                        perf_mode=DR)
                if comp:
                    nc.vector.tensor_scalar_add(
                        dst[:, cb, 0, ssl], pp[:], b_s[:, cb:cb + 1])
                    nc.vector.scalar_tensor_tensor(
                        dst[:, cb, 1, ssl], pp[:], b_s[:, cb:cb + 1],
                        dst[:, cb, 0, ssl], ADD, SUB)
                    nc.sync.dma_start(k8o_s[:, cb, :, ssl],
                                      dst[64:128, cb, :, ssl])
                else:
                    nc.vector.tensor_scalar_add(
                        dst[:, cb, ssl], pp[:], b_s[:, cb:cb + 1])
                    nc.sync.dma_start(q8o_s[:, cb, ssl],
                                      dst[64:128, cb, ssl])

            def emit_projv(kt):
                # V^T tile for k positions [kt*128, (kt+1)*128)
                pp = psA.tile([128, 512], F32, tag="sc", name=f"pv{kt}")
                for hc in range(N_HC):
                    nc.tensor.matmul(
                        pp[:], x16_s[:, hc, kt * 128:(kt + 1) * 128],
                        wvt_s[:, hc, :],
                        start=(hc == 0), stop=(hc == N_HC - 1))
                nc.vector.tensor_tensor(
                    vt_s[:, kt, :, 0:HD],
                    pp[:].rearrange("p (h d) -> p h d", h=HPC), vb_s[:], ADD)

            # ---- attention ----
            # per (qc, kc): QK for 8 heads -> one exp -> 8 PV matmuls.
            # software pipelined one block deep: PV(block i) is emitted after
            # QK(block i+1) so exp(i) overlaps PE work.  PE idle inside the
            # ACT-bound attention loop is filled from a queue of projection /
            # output-projection units.
            pend_q = []   # [(p_tile, kc, qc, accs)] pipeline, depth 2
            acc_of = {}
            fillers = []  # (needed_by_row, emit_fn)

            def emit_norm(qc):
                accs = acc_of.pop(qc)
                ctxT = wp.tile([128, HPC, HD], BF16, tag="ctxT", name=f"cT{qc}")
                for i, a in enumerate(accs):
                    av = a[:, 0:4 * 65].rearrange("p (h e) -> p h e", e=65)
                    denr = wp.tile([128, 4], F32, tag="denr", bufs=4,
                                   name=f"dn{qc}_{i}")
                    nc.vector.reciprocal(denr[:], av[:, :, HD])
                    nc.vector.tensor_tensor(
                        ctxT[:, 4 * i:4 * i + 4, :], av[:, :, 0:HD],
                        denr[:].unsqueeze(2).broadcast_to([128, 4, HD]), MUL)
                # transpose [q, c] -> [c, q] on the DMA crossbar (off PE)
                nc.sync.dma_start_transpose(
                    ctx_s[:, :, qc * 128:(qc + 1) * 128], ctxT[:])
                for oh in range(2):
                    fillers.append((qc + 4, 900.0,
                                    lambda qc=qc, oh=oh: emit_oproj(qc, oh)))

            def flush_pv():
                if not pend_q:
                    return
                p_t, kc, qc, accs = pend_q.pop(0)
                if kc == 0:
                    for a in accs:
                        nc.tensor.matmul(a[:], z1_s[:], z2_s[:],
                                         start=True, stop=True)
                for h in range(HPC):
                    a = accs[h // 4]
                    nc.tensor.matmul(
                        a[:, 65 * (h % 4):65 * (h % 4) + 65],
                        p_t[:, h, :], vt_s[:, kc, h, :],
                        start=False, stop=(kc == qc),
                        skip_group_check=True)
                if kc == qc:
                    emit_norm(qc)

            def emit_qk(qc, kc):
                if kc == 0:
                    accA = psB.tile([128, 512], F32, tag="accA", bufs=1,
                                    name=f"aA{qc}")
                    accB = psB.tile([128, 512], F32, tag="accB", bufs=1,
                                    name=f"aB{qc}")
                    acc_of[qc] = (accA, accB)
                accs = acc_of[qc]
                sc_t = psA.tile([128, HPC, 128], F32, tag="sc",
                                name=f"s{qc}_{kc}")
                qsl = slice(qc * 128, (qc + 1) * 128)
                ksl = slice(kc * 128, (kc + 1) * 128)
                for h in range(HPC):
                    cb = h // 2
                    if h % 2 == 0:
                        k_ap = k8_s[0:64, cb, :, ksl]
                        q_ap = q8_s[0:64, cb, qsl]
                    else:
                        k_ap = k8o_s[:, cb, :, ksl]
                        q_ap = q8o_s[:, cb, qsl]
                    nc.tensor.matmul(
                        sc_t[:, h, :], k_ap,
                        q_ap.unsqueeze(1).broadcast_to([64, 2, 128]),
                        start=True, stop=True, perf_mode=DR)
                p_t = wp.tile([128, HPC, 128], BF16, tag="p", bufs=14,
                              name=f"p{qc}_{kc}")
                nc.scalar.activation(p_t[:], sc_t[:], AF.Exp, scale=float(SCALE))
                if kc == qc:
                    nc.vector.tensor_tensor(
                        p_t[:], p_t[:],
                        ut_s[:].unsqueeze(1).broadcast_to([128, HPC, 128]), MUL)
                pend_q.append((p_t, kc, qc, accs))

            ob_of = {}

            def emit_oproj(sc, oh):
                ssl = slice(sc * 128, (sc + 1) * 128)
                if sc not in ob_of:
                    ob_of[sc] = wp.tile([128, H], BF16, tag="ob",
                                        name=f"ob{sc}")
                ob = ob_of[sc]
                osl = slice(oh * 512, (oh + 1) * 512)
                op = psB.tile([128, 512], F32, tag="op", name=f"o{sc}_{oh}")
                for cb in range(N_CB):
                    nc.tensor.matmul(op[:], ctx_s[:, cb, ssl],
                                     wo_s[:, cb, osl],
                                     start=(cb == 0), stop=(cb == N_CB - 1))
                nc.vector.tensor_copy(ob[:, osl], op[:])
                if oh == 1:
                    nc.sync.dma_start(out_d.ap()[ssl, :], ob[:])
                    del ob_of[sc]

            fill_ns = [0.0]     # filler PE-time emitted so far
            blocks = [0]        # attention blocks emitted so far
            SLACK_NS = 620.0    # PE slack per ACT-bound block

            def pop_fillers(row, opportunistic):
                # emit everything required by this row; then, if opportunistic,
                # fillers up to the cumulative PE-slack budget
                i = 0
                while i < len(fillers):
                    nb, cost, fn = fillers[i]
                    if nb <= row:
                        fillers.pop(i)
                        fill_ns[0] += cost
                        fn()
                    else:
                        i += 1
                slack = 1100.0 if row < 8 else SLACK_NS
                while (opportunistic and fillers
                       and fill_ns[0] < blocks[0] * slack):
                    nb, cost, fn = fillers.pop(0)
                    fill_ns[0] += cost
                    fn()

            # ---- emission schedule ----
            for wu in range(18):
                wt = psB.tile([128, 512], F32, tag="op", name=f"wu{wu}")
                nc.tensor.matmul(wt[:], z1_s[:], z2_s[:], start=True,
                                 stop=True)
            for nb, fn in late_dmas:
                fillers.append((nb, 0.0, fn))
            for st in range(N_ST):
                for cb in range(N_CB):
                    fillers.append((4 * st, 900.0, lambda st=st, cb=cb:
                                    emit_projqk_unit(st, "k", cb)))
                    fillers.append((4 * st, 900.0, lambda st=st, cb=cb:
                                    emit_projqk_unit(st, "q", cb)))
            for kt in range(N_KT):
                fillers.append((kt, 1750.0, lambda kt=kt: emit_projv(kt)))
            fillers.sort(key=lambda f: f[0])

            for qc in range(N_QC):
                pop_fillers(qc, False)   # force units this row needs
                for kc in range(qc + 1):
                    emit_qk(qc, kc)
                    blocks[0] += 1
                    if len(pend_q) > 9:
                        flush_pv()
                    pop_fillers(qc, True)
            while pend_q:
                flush_pv()
            pop_fillers(999, False)
            while fillers:
                fillers.pop(0)[2]()

    nc.compile()
    return nc


def _get_nc():
    if "nc" not in _CACHE:
        _CACHE["nc"] = _build_nc()
    return _CACHE["nc"]


def _e4(a):
    return a.astype(ml_dtypes.float8_e4m3)


def make_in_maps(x, Wq, bq, Wk, bk, Wv, bv, Wo):
    x = np.asarray(x, np.float32)
    in_maps = []
    ut = np.triu(np.ones((128, 128), np.float32)).astype(ml_dtypes.bfloat16)
    idb = np.eye(128, dtype=ml_dtypes.bfloat16)
    for core in range(NCORES):
        b, g = core // 2, core % 2
        csl = slice(g * C, (g + 1) * C)
        xT = np.ascontiguousarray(x[b].T)                       # [H, S]
        x_hi = _e4(xT)
        x_lo = _e4(xT - x_hi.astype(np.float32))
        x8 = np.stack([x_hi.reshape(N_HC, 128, S),
                       x_lo.reshape(N_HC, 128, S)], axis=2)     # [hc,128,2,S]
        x8 = np.ascontiguousarray(x8.transpose(1, 0, 2, 3))     # [128,hc,2,S]
        x16 = np.ascontiguousarray(
            xT.astype(ml_dtypes.bfloat16).reshape(N_HC, 128, S)
            .transpose(1, 0, 2))
        def wlayout(W):
            W8 = _e4(np.asarray(W, np.float32)[csl, :].T)       # [H, C]
            return np.ascontiguousarray(
                W8.reshape(N_HC, 128, C).transpose(1, 0, 2))
        wvt = (np.asarray(Wv, np.float32)[csl, :].T
               .astype(ml_dtypes.bfloat16).reshape(N_HC, 128, C)
               .transpose(1, 0, 2))
        wo = (np.asarray(Wo, np.float32)[:, csl].T
              .astype(ml_dtypes.bfloat16).reshape(N_CB, 128, H)
              .transpose(1, 0, 2))
        vb = np.broadcast_to(
            np.asarray(bv, np.float32)[csl].reshape(HPC, HD), (128, HPC, HD))
        in_maps.append({
            "x8": x8,
            "x16": np.ascontiguousarray(x16),
            "wq8": wlayout(Wq),
            "wk8": wlayout(Wk),
            "wvt": np.ascontiguousarray(wvt),
            "wo": np.ascontiguousarray(wo),
            "bq": np.ascontiguousarray(
                np.asarray(bq, np.float32)[csl].reshape(N_CB, 128).T),
            "bk": np.ascontiguousarray(
                np.asarray(bk, np.float32)[csl].reshape(N_CB, 128).T),
            "vb": np.ascontiguousarray(vb.astype(ml_dtypes.bfloat16)),
            "ut": ut,
            "idb": idb,
        })
    return in_maps


def unshard(partials, bo):
    """partials: list of 8 [S, H] bf16 arrays -> full [B, S, H] f32."""
    out = np.zeros((B, S, H), np.float32)
    for core, p in enumerate(partials):
        out[core // 2] += np.asarray(p, dtype=np.float32)
    return out + np.asarray(bo, np.float32)[None, None, :]


def kernel(x, mask, Wq, bq, Wk, bk, Wv, bv, Wo, bo):
    nc = _get_nc()
    in_maps = make_in_maps(x, Wq, bq, Wk, bk, Wv, bv, Wo)
    res = run_bass_kernel_spmd(nc, in_maps, core_ids=list(range(NCORES)))
    return unshard([r["out"] for r in res.results], bo).astype(np.float32)


if __name__ == "__main__":
    nc = _get_nc()
    from concourse.timeline_sim import TimelineSim
    print("sim ns:", TimelineSim(nc, trace=False).simulate())
